# revision 63
# baseline (speedup 1.0000x reference)
"""Causal multi-head attention with RoPE on 8 Trainium2 NeuronCores.

Tensor-parallel over heads: each core owns 2 of the 16 heads (its slice of
qkv_w rows and o_w columns), computes attention + its partial o_proj for
both batch elements, and the host sums the 8 partial outputs (the
"all-reduce").

Device layout choices (see module builder):
  - q/k computed feature-major [dk, tokens] so scores need no transposes
  - scores computed transposed [k, q]; softmax denominator = ones-matmul
    partition reduction; no max-subtraction (scores are bounded, exp is
    safe in fp32)
  - RoPE via even/odd row-permuted projection weights + a DMA partition
    half-swap + 3 full-width vector ops
  - v kept token-major so attn@v consumes exp(scores) directly
  - all big matmuls in bf16 (1 cycle/row); accumulation in fp32 PSUM
  - y partials stored bf16 (the host all-reduce sums 8 of them in fp64)

Schedule: slot-interleaved. Slot (b, m) emits attention q-group m of batch
b as the "spine" and pumps the NEXT qkv token-group's projection matmuls
plus any ready o_proj tiles into the spine's dependency stalls, so the PE
never sits in a qkv-only (DMA-bound) or attention-only (latency-bound)
phase.
"""

import math
from collections import deque
from contextlib import ExitStack

import numpy as np
import ml_dtypes

D_MODEL = 2048
NUM_HEADS = 16
HEAD_DIM = 128
THETA = 10000.0
B = 2
S = 2048
NCORES = 8
HPC = NUM_HEADS // NCORES  # heads per core
F = HPC * HEAD_DIM  # q/k/v features per core

BF16 = ml_dtypes.bfloat16


def build_module(D=D_MODEL, S_=S, B_=B):
    import concourse.mybir as mybir
    import concourse.tile as tile
    from concourse import bacc, bass_isa
    from concourse.bass import ts, ds

    f32 = mybir.dt.float32
    bf16 = mybir.dt.bfloat16
    Exp = mybir.ActivationFunctionType.Exp

    T = B_ * S_
    DC = D // 128  # contraction chunks
    # weight/x piece boundaries (in dc units). Group 0 (and wq) use a tiny
    # first piece so the very first matmul waits on ~192KB; later groups use
    # even pieces, which pipeline better against their bursty consumption.
    PIECES = [(0, 1), (1, 4), (4, 8), (8, 12), (12, DC)]
    NQ = len(PIECES)
    TG = 512  # token group for qkv projection
    GPB = S_ // TG  # t-groups per batch
    NQG = S_ // 512  # attention q-groups per batch
    NVC = T // 128  # v token chunks
    KPB = S_ // 128  # k tiles per batch
    scale = 1.0 / math.sqrt(HEAD_DIM)

    nc = bacc.Bacc("TRN2", target_bir_lowering=False, debug=False)

    xt = nc.dram_tensor("xt", [D, T], bf16, kind="ExternalInput")[:]
    wq = nc.dram_tensor("wq", [D, F], bf16, kind="ExternalInput")[:]
    wk = nc.dram_tensor("wk", [D, F], bf16, kind="ExternalInput")[:]
    wv = nc.dram_tensor("wv", [D, F], bf16, kind="ExternalInput")[:]
    wo = nc.dram_tensor("wo", [F, D], bf16, kind="ExternalInput")[:]
    cs2 = nc.dram_tensor("cs2", [128, S_], bf16, kind="ExternalInput")[:]
    sn2 = nc.dram_tensor("sn2", [128, S_], bf16, kind="ExternalInput")[:]
    mk = nc.dram_tensor("mk", [128, 512], bf16, kind="ExternalInput")[:]
    y = nc.dram_tensor("y", [T, D], bf16, kind="ExternalOutput")[:]

    with tile.TileContext(nc) as tc:
        with ExitStack() as ctx:
            const = ctx.enter_context(tc.tile_pool(name="const", bufs=1))
            xpool = ctx.enter_context(tc.tile_pool(name="xpool", bufs=2))
            store = ctx.enter_context(tc.tile_pool(name="store", bufs=1))
            rope = ctx.enter_context(tc.tile_pool(name="rope", bufs=2))
            ypool = ctx.enter_context(tc.tile_pool(name="ypool", bufs=4))
            epool = ctx.enter_context(tc.tile_pool(name="epool", bufs=8))
            spool = ctx.enter_context(tc.tile_pool(name="spool", bufs=2))
            psum = ctx.enter_context(tc.tile_pool(name="psum", bufs=3, space="PSUM"))
            psgen = ctx.enter_context(tc.tile_pool(name="psgen", bufs=3, space="PSUM"))
            psacc = ctx.enter_context(tc.tile_pool(name="psacc", bufs=2, space="PSUM"))

            # ---- constants ----
            # wq is split into pieces with independent semaphores so the very
            # first matmul only waits for its piece (+ the first xg piece),
            # not the whole weight tile.
            wq_r = wq.rearrange("(o p) f -> p o f", p=128)
            wq_t = [
                const.tile([128, p1 - p0, F], bf16, tag=f"wq{s}", name=f"wq_t{s}")
                for s, (p0, p1) in enumerate(PIECES)
            ]

            def piece_of(dc, pieces):
                for s, (p0, p1) in enumerate(pieces):
                    if p0 <= dc < p1:
                        return s, dc - p0
                raise AssertionError(dc)
            wk_sb = const.tile([128, DC, F], bf16, tag="wk")
            wv_sb = const.tile([128, DC, F], bf16, tag="wv")
            wo_sb = const.tile([128, HPC, D], bf16, tag="wo")
            cs2_sb = const.tile([128, S_], bf16, tag="cs2")
            sn2_sb = const.tile([128, S_], bf16, tag="sn2")
            mk_sb = const.tile([128, 512], bf16, tag="mk")

            def wq_at(dc):
                s, o = piece_of(dc, PIECES)
                return wq_t[s][:, o, :]

            def wk_at(dc):
                return wk_sb[:, dc, :]

            def emit_late_consts():
                # scalar-ring order tuned to consumption times: wk first
                # (k-ftile matmuls need it ~14us in), then the block-0 rope
                # tables (first rope ~15us), then wv (~19us) and the causal
                # mask (~22us)
                nc.scalar.dma_start(
                    out=wk_sb[:], in_=wk.rearrange("(o p) f -> p o f", p=128)
                )
                nc.scalar.dma_start(out=cs2_sb[:, 0:TG], in_=cs2[:, 0:TG])
                nc.scalar.dma_start(out=sn2_sb[:, 0:TG], in_=sn2[:, 0:TG])
                nc.scalar.dma_start(
                    out=wv_sb[:], in_=wv.rearrange("(o p) f -> p o f", p=128)
                )
                nc.scalar.dma_start(out=mk_sb[:], in_=mk[:, :])

            # ---- persistent stores ----
            q_sb = store.tile([128, HPC, T], bf16, tag="q")  # [dk, ht, tok]
            k_sb = store.tile([128, HPC, T], bf16, tag="k")
            v_sb = store.tile([128, NVC, F], bf16, tag="v")  # [tok128, chunk, f]
            ao_sb = store.tile([128, HPC, T], bf16, tag="ao")  # [dk, ht, tok]

            xt_r = xt.rearrange("(o p) t -> p o t", p=128)

            qk_jobs = [
                (wq_at, q_sb, 0),
                (wq_at, q_sb, 1),
                (wk_at, k_sb, 0),
                (wk_at, k_sb, 1),
            ]
            # per-group quarter tiles: xg_tiles[g] = list of NQ tiles
            xg_tiles = {}

            def emit_xg(g, interleave_wq=False):
                # each piece is its own tile (own semaphore) so consumers
                # wake up per-piece
                parts = []
                for s_, (p0, p1) in enumerate(PIECES):
                    xq = xpool.tile(
                        [128, p1 - p0, TG],
                        bf16,
                        tag=f"xg{s_}",
                        name=f"xg_{g}_{s_}",
                    )
                    if interleave_wq:
                        # wq rides the otherwise-idle gpsimd ring: packets of
                        # queued transfers complete interleaved within a ring,
                        # so sharing one ring would delay every piece's
                        # completion semaphore
                        nc.gpsimd.dma_start(
                            out=wq_t[s_][:], in_=wq_r[:, ds(p0, p1 - p0), :]
                        )
                    nc.sync.dma_start(
                        out=xq[:], in_=xt_r[:, ds(p0, p1 - p0), ts(g, TG)]
                    )
                    parts.append(xq)
                xg_tiles[g] = parts

            def xg_at(g, dc):
                s, o = piece_of(dc, PIECES)
                return xg_tiles[g][s][:, o, :]

            def emit_qk_ftile(g, job):
                w_at, qk_store, ht = job
                pos_sl = ds((g % GPB) * TG, TG)
                ps = psgen.tile([128, TG], f32, tag="gen")
                for dc in range(DC):
                    nc.tensor.matmul(
                        ps[:],
                        lhsT=w_at(dc)[:, ts(ht, 128)],
                        rhs=xg_at(g, dc),
                        start=(dc == 0),
                        stop=(dc == DC - 1),
                    )
                # rope: rot = raw*cos2 + halfswap(raw*sin2_pre); bf16
                # intermediates: the final add runs in the DVE 2x mode and
                # the half-swap DMA halves
                t0 = rope.tile([128, TG], bf16, tag="t0")
                nc.vector.tensor_mul(out=t0[:], in0=ps[:], in1=cs2_sb[:, pos_sl])
                t1s = rope.tile([128, TG], bf16, tag="t1s")
                nc.vector.tensor_mul(out=t1s[:], in0=ps[:], in1=sn2_sb[:, pos_sl])
                # gpsimd software-DGE ring: keeps these latency-critical swaps
                # off the sync (xg) and scalar (consts) rings, and costs the
                # gpsimd engine almost nothing to issue
                t1w = rope.tile([128, TG], bf16, tag="t1w")
                nc.gpsimd.dma_start(out=t1w[0:64, :], in_=t1s[64:128, :])
                nc.gpsimd.dma_start(out=t1w[64:128, :], in_=t1s[0:64, :])
                nc.vector.tensor_add(
                    out=qk_store[:, ht, ts(g, TG)], in0=t0[:], in1=t1w[:]
                )

            def emit_v_sub(g, tsub):
                psv = psgen.tile([128, F], f32, tag="gen")
                for dc in range(DC):
                    nc.tensor.matmul(
                        psv[:],
                        lhsT=xg_at(g, dc)[:, ts(tsub, 128)],
                        rhs=wv_sb[:, dc, :],
                        start=(dc == 0),
                        stop=(dc == DC - 1),
                    )
                nc.scalar.copy(out=v_sb[:, g * (TG // 128) + tsub, :], in_=psv[:])

            def emit_oproj_eg(tt, eg, pool=None):
                if pool is None:
                    yp = psgen.tile([128, 512], f32, tag="gen", name="yp")
                elif pool is psacc:
                    yp = pool.tile([128, 512], f32, tag="acc", name="yp")
                else:
                    yp = pool.tile([128, 512], f32, tag="work", name="yp")
                for ht in range(HPC):
                    nc.tensor.matmul(
                        yp[:],
                        lhsT=ao_sb[:, ht, ts(tt, 128)],
                        rhs=wo_sb[:, ht, ts(eg, 512)],
                        start=(ht == 0),
                        stop=(ht == HPC - 1),
                    )
                # bf16 partial-sum chunk, drained immediately on the gpsimd
                # ring so y writeback never queues behind xg loads
                yc = ypool.tile([128, 512], bf16, tag="ysb")
                if eg % 2 == 0:
                    nc.vector.tensor_copy(out=yc[:], in_=yp[:])
                else:
                    nc.scalar.copy(out=yc[:], in_=yp[:])
                nc.gpsimd.dma_start(out=y[ts(tt, 128), ts(eg, 512)], in_=yc[:])

            def emit_oproj_tile(tt, pool=None):
                for eg in range(D // 512):
                    emit_oproj_eg(tt, eg, pool=pool)

            # ---- filler queues ----
            # hard: next qkv group's jobs — MUST fully emit within the slot
            # (the next slot's attention reads them).  soft: ready o_proj
            # tiles — best-effort, drained at a steady pace.
            hardq = deque()
            softq = deque()

            LOOKAHEAD = 2

            pending = [None]

            def do_pending():
                if pending[0] is not None:
                    pending[0]()
                    pending[0] = None

            def emit_attention_slot(
                b, m, post_group_hook=None, pump_hard=True, soft_reserve=0,
                pump_first=0,
            ):
                # run the previous slot's deferred finalize before this
                # slot's streams touch the rotating ao/sacc buffers it reads
                do_pending()
                for _ in range(pump_first):
                    if hardq:
                        hardq.popleft()()
                nk = 4 * m + 4
                total_steps = HPC * nk
                st = {"done": 0, "hacc": 0.0, "sacc": 0.0}

                # pace the hard queue to finish a few steps early so the
                # qkv->rope->store chain completes before the next slot's
                # attention reads it.  pump_hard=False (slot 0): its xg is
                # still arriving — pumping would head-of-line-block the PE
                # behind DMA while ready attention work sits queued.
                margin = 0

                def paced_pump():
                    st["done"] += 1
                    rem = max(1, total_steps - st["done"] - margin)
                    if pump_hard:
                        st["hacc"] += len(hardq) / rem
                        n = int(st["hacc"])
                        if n:
                            st["hacc"] -= n
                            for _ in range(n):
                                if hardq:
                                    hardq.popleft()()
                    st["sacc"] += max(0, len(softq) - soft_reserve) / rem
                    n = min(2, int(st["sacc"]))
                    if n:
                        st["sacc"] -= n
                        for _ in range(n):
                            if softq:
                                softq.popleft()()

                # the two heads' blocks are interleaved j-by-j: the other
                # head's matmuls hide each head's exp/mask/sacc chain, so the
                # PE keeps streaming even when the filler queues run thin
                streams = []
                for ht in range(HPC):
                    streams.append(
                        {
                            "ht": ht,
                            "qv": q_sb[:, ht, ds(b * S_, S_)],
                            "kv": k_sb[:, ht, ds(b * S_, S_)],
                            "ao": psacc.tile(
                                [128, 512], f32, tag="acc", name=f"ao{ht}"
                            ),
                            # bf16 accumulator: all-bf16 adds run in the DVE
                            # 2x mode; the denominator already went through
                            # bf16 for the ones-matmul anyway
                            "sacc": spool.tile(
                                [128, 512], bf16, tag="sacc", name=f"sacc{ht}"
                            ),
                            "eTs": {},
                        }
                    )

                def emit_scores(s, j):
                    p = j - 4 * m  # >= 0 on diagonal superblock
                    col0 = max(0, p * 128)
                    ncol = 512 - col0
                    s_ps = psum.tile([128, 512], f32, tag="work")
                    nc.tensor.matmul(
                        s_ps[:, col0:],
                        lhsT=s["kv"][:, ts(j, 128)],
                        rhs=s["qv"][:, ds(m * 512 + col0, ncol)],
                        start=True,
                        stop=True,
                    )
                    eT = epool.tile([128, 512], bf16, tag="eT")
                    nc.scalar.activation(
                        out=eT[:, col0:], in_=s_ps[:, col0:], func=Exp, scale=scale
                    )
                    if p >= 0:
                        nc.vector.tensor_mul(
                            out=eT[:, col0:],
                            in0=eT[:, col0:],
                            in1=mk_sb[:, 0:ncol],
                        )
                    if j == 0:
                        nc.vector.tensor_copy(out=s["sacc"][:], in_=eT[:])
                    else:
                        nc.vector.tensor_add(
                            out=s["sacc"][:, col0:],
                            in0=s["sacc"][:, col0:],
                            in1=eT[:, col0:],
                        )
                    s["eTs"][j] = (eT, col0)

                def emit_attnv(s, j):
                    eT, col0 = s["eTs"].pop(j)
                    nc.tensor.matmul(
                        s["ao"][:, col0:],
                        lhsT=v_sb[:, b * KPB + j, ts(s["ht"], 128)],
                        rhs=eT[:, col0:],
                        start=(j == 0),
                        stop=(j == nk - 1),
                    )

                for j in range(nk):
                    for s in streams:
                        emit_scores(s, j)
                        if j >= LOOKAHEAD:
                            emit_attnv(s, j - LOOKAHEAD)
                        paced_pump()
                for j in range(max(0, nk - LOOKAHEAD), nk):
                    for s in streams:
                        emit_attnv(s, j)

                def make_finalize(s):
                    def finalize(b=b, m=m):
                        # softmax denominator: partition reduction on the
                        # (mostly idle) gpsimd engine — frees ~7us of PE
                        # ones-matmuls and the PSUM bank they accumulated in
                        rps = spool.tile([128, 512], f32, tag="rps")
                        nc.gpsimd.partition_all_reduce(
                            rps[:],
                            s["sacc"][:],
                            channels=128,
                            reduce_op=bass_isa.ReduceOp.add,
                        )
                        rsb = spool.tile([128, 512], f32, tag="rsb")
                        nc.vector.reciprocal_approx_fast(out=rsb[:], in_=rps[:])
                        # normalize in 128-column chunks: each downstream
                        # o_proj tile only reads its own 128 tokens, so it can
                        # start as soon as its chunk lands instead of waiting
                        # for the full 512-wide multiply
                        for c4 in range(4):
                            nc.vector.tensor_mul(
                                out=ao_sb[
                                    :, s["ht"], ds(b * S_ + m * 512 + c4 * 128, 128)
                                ],
                                in0=s["ao"][:, ts(c4, 128)],
                                in1=rsb[:, ts(c4, 128)],
                            )
                        if s["ht"] == HPC - 1 and post_group_hook is not None:
                            post_group_hook(m)

                    return finalize

                # first head's finalize lands here (the tail of the other
                # head's work hides its DVE burst); the last head's is
                # deferred into the next slot as before
                make_finalize(streams[0])()
                pending[0] = make_finalize(streams[1])

                # hard filler must be fully emitted before the next slot's
                # attention (which reads the q/k/v it produces)
                while hardq:
                    hardq.popleft()()

            # ---- program order ----
            emit_xg(0, interleave_wq=True)
            emit_late_consts()
            # dense first group: nothing can overlap it (everything depends
            # on x arriving)
            for job in qk_jobs:
                emit_qk_ftile(0, job)
            for tsub in range(TG // 128):
                emit_v_sub(0, tsub)
            emit_xg(1)
            if GPB > 1:
                # remaining rope-table blocks ride the gpsimd ring (idle
                # after the wq pieces) — on the sync ring they would delay
                # xg(2) behind them; needed by group-1 ftiles in slot 0
                nc.gpsimd.dma_start(out=cs2_sb[:, TG:], in_=cs2[:, TG:])
                nc.gpsimd.dma_start(out=sn2_sb[:, TG:], in_=sn2[:, TG:])
            # wo is first needed by o_proj of group 0 (~45us in): keep it out
            # of the congested early-DMA window
            nc.gpsimd.dma_start(
                out=wo_sb[:], in_=wo.rearrange("(o p) e -> p o e", p=128)
            )

            def queue_qkv_group(g):
                for job in qk_jobs:
                    hardq.append(lambda job=job: emit_qk_ftile(g, job))
                for tsub in range(TG // 128):
                    hardq.append(lambda tsub=tsub: emit_v_sub(g, tsub))

            def oproj_hook(b):
                def hook(m):
                    base = b * (T // 256)
                    for tt in range(base + 4 * m, base + 4 * m + 4):
                        softq.append(
                            lambda tt=tt, pool=None: emit_oproj_tile(tt, pool=pool)
                        )

                return hook

            NSLOT = B_ * NQG
            for si in range(NSLOT):
                b, m = divmod(si, NQG)
                # global qkv group consumed by slot si is g_next = si + 1
                g_next = si + 1
                if g_next < B_ * GPB:
                    queue_qkv_group(g_next)
                if g_next + 1 < B_ * GPB:
                    emit_xg(g_next + 1)
                emit_attention_slot(
                    b,
                    m,
                    post_group_hook=oproj_hook(b),
                    soft_reserve=2 if si == NSLOT - 1 else 0,
                    pump_first=2 if (si > 0 and m == 0) else 0,
                )

            do_pending()
            # tail drain: the last group's tiles (tail of softq) depend on the
            # just-emitted finalize; lead with the reserved independent tiles
            # and alternate PSUM pools (the scores pool is idle now)
            tiles = [softq.popleft() for _ in range(len(softq))]
            indep, dep = tiles[:-4], tiles[-4:]
            order = []
            while indep or dep:
                if indep:
                    order.append(indep.pop(0))
                if dep:
                    order.append(dep.pop(0))
            for ti, fn in enumerate(order):
                fn(pool=[None, psum, psacc][ti % 3])

    nc.compile()
    return nc


def _rope_tables(token_positions, S_):
    pos = np.asarray(token_positions).astype(np.float32)
    dim_id = np.arange(0, HEAD_DIM, 2, dtype=np.float32)
    inv_freq = np.power(np.float32(THETA), dim_id / np.float32(HEAD_DIM)).astype(
        np.float32
    )
    ang = (pos[None, :] / inv_freq[:, None]).astype(np.float32)  # [64, S]
    cos = np.cos(ang).astype(np.float32)
    sin = np.sin(ang).astype(np.float32)
    cs2 = np.concatenate([cos, cos], axis=0)  # [128, S]
    # pre-multiply sign layout: top half (x0 rows) gets +sin (feeds r1 after
    # the half-swap), bottom half (x1 rows) gets -sin (feeds r0)
    sn2 = np.concatenate([sin, -sin], axis=0)
    return np.ascontiguousarray(cs2), np.ascontiguousarray(sn2)


def _masks():
    # single lower-triangle table: for a diagonal tile at k-offset p*128 the
    # matmul already starts at column col0 = p*128, so the mask seen by the
    # surviving columns is always (k <= q - col0)
    kl = np.arange(128)[:, None]
    ql = np.arange(512)[None, :]
    return np.ascontiguousarray((kl <= ql).astype(np.float32).astype(BF16))


def _perm(n_heads):
    # within each 128-feature head block: evens then odds
    p = []
    for h in range(n_heads):
        base = h * HEAD_DIM
        p.extend(range(base, base + HEAD_DIM, 2))
        p.extend(range(base + 1, base + HEAD_DIM, 2))
    return np.array(p, dtype=np.int64)


def prepare_in_maps(x, token_positions, qkv_w, o_w, D=D_MODEL, S_=S, B_=B, ncores=NCORES):
    T = B_ * S_
    x = np.asarray(x, dtype=np.float32)
    qkv_w = np.asarray(qkv_w, dtype=np.float32)
    o_w = np.asarray(o_w, dtype=np.float32)

    xt = np.ascontiguousarray(x.reshape(T, D).T).astype(BF16)  # [D, T]
    cs2, sn2 = _rope_tables(token_positions, S_)
    cs2 = cs2.astype(BF16)
    sn2 = sn2.astype(BF16)
    mk = _masks()
    perm = _perm(HPC)

    in_maps = []
    for c in range(ncores):
        r0 = c * F
        qrows = qkv_w[r0 : r0 + F]
        krows = qkv_w[D + r0 : D + r0 + F]
        vrows = qkv_w[2 * D + r0 : 2 * D + r0 + F]
        wq_c = np.ascontiguousarray(qrows[perm].T).astype(BF16)  # [D, F]
        wk_c = np.ascontiguousarray(krows[perm].T).astype(BF16)
        wv_c = np.ascontiguousarray(vrows.T).astype(BF16)
        wo_c = np.ascontiguousarray(o_w[:, r0 : r0 + F].T).astype(BF16)  # [F, D]
        in_maps.append(
            {
                "xt": xt,
                "wq": wq_c,
                "wk": wk_c,
                "wv": wv_c,
                "wo": wo_c,
                "cs2": cs2,
                "sn2": sn2,
                "mk": mk,
            }
        )
    return in_maps


_CACHE = {}


def kernel_with_results(x, token_positions, qkv_w, o_w, trace=False, **kw):
    from concourse.bass_utils import run_bass_kernel_spmd

    if "nc" not in _CACHE:
        _CACHE["nc"] = build_module()
    nc = _CACHE["nc"]

    in_maps = prepare_in_maps(x, token_positions, qkv_w, o_w)
    res = run_bass_kernel_spmd(
        nc, in_maps, core_ids=list(range(NCORES)), trace=trace, **kw
    )
    acc = np.zeros((B * S, D_MODEL), dtype=np.float64)
    for r in res.results:
        acc += r["y"].astype(np.float64)
    return acc.astype(np.float32).reshape(B, S, D_MODEL), res


def kernel(x, token_positions, qkv_w, o_w):
    out, _ = kernel_with_results(x, token_positions, qkv_w, o_w)
    return out


# revision 64
# speedup vs baseline: 1.1527x; 1.1527x over previous
"""Causal multi-head attention with RoPE on 8 Trainium2 NeuronCores.

Tensor-parallel over heads: each core owns 2 of the 16 heads (its slice of
qkv_w rows and o_w columns), computes attention + its partial o_proj for
both batch elements, and the host sums the 8 partial outputs (the
"all-reduce").

Device layout choices (see module builder):
  - q/k computed feature-major [dk, tokens] so scores need no transposes
  - scores computed transposed [k, q]; softmax denominator = ones-matmul
    partition reduction; no max-subtraction (scores are bounded, exp is
    safe in fp32)
  - RoPE via even/odd row-permuted projection weights + a DMA partition
    half-swap + 3 full-width vector ops
  - v kept token-major so attn@v consumes exp(scores) directly
  - all big matmuls in bf16 (1 cycle/row); accumulation in fp32 PSUM
  - y partials stored bf16 (the host all-reduce sums 8 of them in fp64)

Schedule: slot-interleaved. Slot (b, m) emits attention q-group m of batch
b as the "spine" and pumps the NEXT qkv token-group's projection matmuls
plus any ready o_proj tiles into the spine's dependency stalls, so the PE
never sits in a qkv-only (DMA-bound) or attention-only (latency-bound)
phase.
"""

import math
from collections import deque
from contextlib import ExitStack

import numpy as np
import ml_dtypes

D_MODEL = 2048
NUM_HEADS = 16
HEAD_DIM = 128
THETA = 10000.0
B = 2
S = 2048
NCORES = 8
HPC = NUM_HEADS // NCORES  # heads per core
F = HPC * HEAD_DIM  # q/k/v features per core

BF16 = ml_dtypes.bfloat16


def build_module(D=D_MODEL, S_=S, B_=B):
    import concourse.mybir as mybir
    import concourse.tile as tile
    from concourse import bacc
    from concourse.bass import ts, ds

    f32 = mybir.dt.float32
    bf16 = mybir.dt.bfloat16
    Exp = mybir.ActivationFunctionType.Exp

    T = B_ * S_
    DC = D // 128  # contraction chunks
    # weight/x piece boundaries (in dc units). Group 0 (and wq) use a tiny
    # first piece so the very first matmul waits on ~192KB; later groups use
    # even pieces, which pipeline better against their bursty consumption.
    PIECES = [(0, 1), (1, 4), (4, 8), (8, 12), (12, DC)]
    NQ = len(PIECES)
    TG = 512  # token group for qkv projection
    GPB = S_ // TG  # t-groups per batch
    NQG = S_ // 512  # attention q-groups per batch
    NVC = T // 128  # v token chunks
    KPB = S_ // 128  # k tiles per batch
    scale = 1.0 / math.sqrt(HEAD_DIM)

    nc = bacc.Bacc("TRN2", target_bir_lowering=False, debug=False)

    xt = nc.dram_tensor("xt", [D, T], bf16, kind="ExternalInput")[:]
    wq = nc.dram_tensor("wq", [D, F], bf16, kind="ExternalInput")[:]
    wk = nc.dram_tensor("wk", [D, F], bf16, kind="ExternalInput")[:]
    wv = nc.dram_tensor("wv", [D, F], bf16, kind="ExternalInput")[:]
    wo = nc.dram_tensor("wo", [F, D], bf16, kind="ExternalInput")[:]
    cs2 = nc.dram_tensor("cs2", [128, S_], bf16, kind="ExternalInput")[:]
    sn2 = nc.dram_tensor("sn2", [128, S_], bf16, kind="ExternalInput")[:]
    mk = nc.dram_tensor("mk", [128, 512], bf16, kind="ExternalInput")[:]
    y = nc.dram_tensor("y", [T, D], bf16, kind="ExternalOutput")[:]

    with tile.TileContext(nc) as tc:
        with ExitStack() as ctx:
            const = ctx.enter_context(tc.tile_pool(name="const", bufs=1))
            xpool = ctx.enter_context(tc.tile_pool(name="xpool", bufs=2))
            store = ctx.enter_context(tc.tile_pool(name="store", bufs=1))
            rope = ctx.enter_context(tc.tile_pool(name="rope", bufs=2))
            ypool = ctx.enter_context(tc.tile_pool(name="ypool", bufs=4))
            epool = ctx.enter_context(tc.tile_pool(name="epool", bufs=8))
            spool = ctx.enter_context(tc.tile_pool(name="spool", bufs=2))
            psum = ctx.enter_context(tc.tile_pool(name="psum", bufs=3, space="PSUM"))
            psgen = ctx.enter_context(tc.tile_pool(name="psgen", bufs=2, space="PSUM"))
            psacc = ctx.enter_context(tc.tile_pool(name="psacc", bufs=2, space="PSUM"))
            psred = ctx.enter_context(tc.tile_pool(name="psred", bufs=1, space="PSUM"))

            # ---- constants ----
            # wq is split into pieces with independent semaphores so the very
            # first matmul only waits for its piece (+ the first xg piece),
            # not the whole weight tile.
            wq_r = wq.rearrange("(o p) f -> p o f", p=128)
            wq_t = [
                const.tile([128, p1 - p0, F], bf16, tag=f"wq{s}", name=f"wq_t{s}")
                for s, (p0, p1) in enumerate(PIECES)
            ]

            def piece_of(dc, pieces):
                for s, (p0, p1) in enumerate(pieces):
                    if p0 <= dc < p1:
                        return s, dc - p0
                raise AssertionError(dc)
            wk_sb = const.tile([128, DC, F], bf16, tag="wk")
            wv_sb = const.tile([128, DC, F], bf16, tag="wv")
            wo_sb = const.tile([128, HPC, D], bf16, tag="wo")
            cs2_sb = const.tile([128, S_], bf16, tag="cs2")
            sn2_sb = const.tile([128, S_], bf16, tag="sn2")
            mk_sb = const.tile([128, 512], bf16, tag="mk")
            ones_sb = const.tile([128, 128], bf16, tag="ones")

            def wq_at(dc):
                s, o = piece_of(dc, PIECES)
                return wq_t[s][:, o, :]

            def wk_at(dc):
                return wk_sb[:, dc, :]

            def emit_late_consts():
                # scalar-ring order tuned to consumption times: wk first
                # (k-ftile matmuls need it ~14us in), then the block-0 rope
                # tables (first rope ~15us), then wv (~19us) and the causal
                # mask (~22us)
                nc.scalar.dma_start(
                    out=wk_sb[:], in_=wk.rearrange("(o p) f -> p o f", p=128)
                )
                nc.scalar.dma_start(out=cs2_sb[:, 0:TG], in_=cs2[:, 0:TG])
                nc.scalar.dma_start(out=sn2_sb[:, 0:TG], in_=sn2[:, 0:TG])
                nc.scalar.dma_start(
                    out=wv_sb[:], in_=wv.rearrange("(o p) f -> p o f", p=128)
                )
                nc.scalar.dma_start(out=mk_sb[:], in_=mk[:, :])
                nc.vector.memset(ones_sb[:], 1.0)

            # ---- persistent stores ----
            q_sb = store.tile([128, HPC, T], bf16, tag="q")  # [dk, ht, tok]
            k_sb = store.tile([128, HPC, T], bf16, tag="k")
            v_sb = store.tile([128, NVC, F], bf16, tag="v")  # [tok128, chunk, f]
            ao_sb = store.tile([128, HPC, T], bf16, tag="ao")  # [dk, ht, tok]

            xt_r = xt.rearrange("(o p) t -> p o t", p=128)

            qk_jobs = [
                (wq_at, q_sb, 0),
                (wq_at, q_sb, 1),
                (wk_at, k_sb, 0),
                (wk_at, k_sb, 1),
            ]
            # per-group quarter tiles: xg_tiles[g] = list of NQ tiles
            xg_tiles = {}

            def emit_xg(g, interleave_wq=False):
                # each piece is its own tile (own semaphore) so consumers
                # wake up per-piece
                parts = []
                for s_, (p0, p1) in enumerate(PIECES):
                    xq = xpool.tile(
                        [128, p1 - p0, TG],
                        bf16,
                        tag=f"xg{s_}",
                        name=f"xg_{g}_{s_}",
                    )
                    if interleave_wq:
                        # wq rides the otherwise-idle gpsimd ring: packets of
                        # queued transfers complete interleaved within a ring,
                        # so sharing one ring would delay every piece's
                        # completion semaphore
                        nc.gpsimd.dma_start(
                            out=wq_t[s_][:], in_=wq_r[:, ds(p0, p1 - p0), :]
                        )
                    nc.sync.dma_start(
                        out=xq[:], in_=xt_r[:, ds(p0, p1 - p0), ts(g, TG)]
                    )
                    parts.append(xq)
                xg_tiles[g] = parts

            def xg_at(g, dc):
                s, o = piece_of(dc, PIECES)
                return xg_tiles[g][s][:, o, :]

            def emit_qk_ftile(g, job):
                w_at, qk_store, ht = job
                pos_sl = ds((g % GPB) * TG, TG)
                ps = psgen.tile([128, TG], f32, tag="gen")
                for dc in range(DC):
                    nc.tensor.matmul(
                        ps[:],
                        lhsT=w_at(dc)[:, ts(ht, 128)],
                        rhs=xg_at(g, dc),
                        start=(dc == 0),
                        stop=(dc == DC - 1),
                    )
                # rope: rot = raw*cos2 + halfswap(raw*sin2_pre); bf16
                # intermediates: the final add runs in the DVE 2x mode and
                # the half-swap DMA halves
                t0 = rope.tile([128, TG], bf16, tag="t0")
                nc.vector.tensor_mul(out=t0[:], in0=ps[:], in1=cs2_sb[:, pos_sl])
                t1s = rope.tile([128, TG], bf16, tag="t1s")
                nc.vector.tensor_mul(out=t1s[:], in0=ps[:], in1=sn2_sb[:, pos_sl])
                # gpsimd software-DGE ring: keeps these latency-critical swaps
                # off the sync (xg) and scalar (consts) rings, and costs the
                # gpsimd engine almost nothing to issue
                t1w = rope.tile([128, TG], bf16, tag="t1w")
                nc.gpsimd.dma_start(out=t1w[0:64, :], in_=t1s[64:128, :])
                nc.gpsimd.dma_start(out=t1w[64:128, :], in_=t1s[0:64, :])
                nc.vector.tensor_add(
                    out=qk_store[:, ht, ts(g, TG)], in0=t0[:], in1=t1w[:]
                )

            def emit_v_sub(g, tsub):
                psv = psgen.tile([128, F], f32, tag="gen")
                for dc in range(DC):
                    nc.tensor.matmul(
                        psv[:],
                        lhsT=xg_at(g, dc)[:, ts(tsub, 128)],
                        rhs=wv_sb[:, dc, :],
                        start=(dc == 0),
                        stop=(dc == DC - 1),
                    )
                nc.scalar.copy(out=v_sb[:, g * (TG // 128) + tsub, :], in_=psv[:])

            def emit_oproj_eg(tt, eg, pool=None):
                if pool is None:
                    yp = psgen.tile([128, 512], f32, tag="gen", name="yp")
                elif pool is psacc:
                    yp = pool.tile([128, 512], f32, tag="acc", name="yp")
                else:
                    yp = pool.tile([128, 512], f32, tag="work", name="yp")
                for ht in range(HPC):
                    nc.tensor.matmul(
                        yp[:],
                        lhsT=ao_sb[:, ht, ts(tt, 128)],
                        rhs=wo_sb[:, ht, ts(eg, 512)],
                        start=(ht == 0),
                        stop=(ht == HPC - 1),
                    )
                # bf16 partial-sum chunk, drained immediately on the gpsimd
                # ring so y writeback never queues behind xg loads
                yc = ypool.tile([128, 512], bf16, tag="ysb")
                if eg % 2 == 0:
                    nc.vector.tensor_copy(out=yc[:], in_=yp[:])
                else:
                    nc.scalar.copy(out=yc[:], in_=yp[:])
                nc.gpsimd.dma_start(out=y[ts(tt, 128), ts(eg, 512)], in_=yc[:])

            def emit_oproj_tile(tt, pool=None):
                for eg in range(D // 512):
                    emit_oproj_eg(tt, eg, pool=pool)

            # ---- filler queues ----
            # hard: next qkv group's jobs — MUST fully emit within the slot
            # (the next slot's attention reads them).  soft: ready o_proj
            # tiles — best-effort, drained at a steady pace.
            hardq = deque()
            softq = deque()

            LOOKAHEAD = 2

            pending = [None]

            def do_pending():
                if pending[0] is not None:
                    pending[0]()
                    pending[0] = None

            def emit_attention_slot(
                b, m, post_group_hook=None, pump_hard=True, soft_reserve=0,
                pump_first=0,
            ):
                # run the previous slot's deferred finalize before this
                # slot's streams touch the rotating ao/sacc buffers it reads
                do_pending()
                for _ in range(pump_first):
                    if hardq:
                        hardq.popleft()()
                nk = 4 * m + 4
                total_steps = HPC * nk
                st = {"done": 0, "hacc": 0.0, "sacc": 0.0}

                # pace the hard queue to finish a few steps early so the
                # qkv->rope->store chain completes before the next slot's
                # attention reads it.  pump_hard=False (slot 0): its xg is
                # still arriving — pumping would head-of-line-block the PE
                # behind DMA while ready attention work sits queued.
                margin = 0

                def paced_pump():
                    st["done"] += 1
                    rem = max(1, total_steps - st["done"] - margin)
                    if pump_hard:
                        st["hacc"] += len(hardq) / rem
                        n = int(st["hacc"])
                        if n:
                            st["hacc"] -= n
                            for _ in range(n):
                                if hardq:
                                    hardq.popleft()()
                    st["sacc"] += max(0, len(softq) - soft_reserve) / rem
                    n = min(2, int(st["sacc"]))
                    if n:
                        st["sacc"] -= n
                        for _ in range(n):
                            if softq:
                                softq.popleft()()

                # the two heads' blocks are interleaved j-by-j: the other
                # head's matmuls hide each head's exp/mask/sacc chain, so the
                # PE keeps streaming even when the filler queues run thin
                streams = []
                for ht in range(HPC):
                    streams.append(
                        {
                            "ht": ht,
                            "qv": q_sb[:, ht, ds(b * S_, S_)],
                            "kv": k_sb[:, ht, ds(b * S_, S_)],
                            "ao": psacc.tile(
                                [128, 512], f32, tag="acc", name=f"ao{ht}"
                            ),
                            # bf16 accumulator: all-bf16 adds run in the DVE
                            # 2x mode; the denominator already went through
                            # bf16 for the ones-matmul anyway
                            "sacc": spool.tile(
                                [128, 512], bf16, tag="sacc", name=f"sacc{ht}"
                            ),
                            "eTs": {},
                        }
                    )

                def emit_scores(s, j):
                    p = j - 4 * m  # >= 0 on diagonal superblock
                    col0 = max(0, p * 128)
                    ncol = 512 - col0
                    s_ps = psum.tile([128, 512], f32, tag="work")
                    nc.tensor.matmul(
                        s_ps[:, col0:],
                        lhsT=s["kv"][:, ts(j, 128)],
                        rhs=s["qv"][:, ds(m * 512 + col0, ncol)],
                        start=True,
                        stop=True,
                    )
                    eT = epool.tile([128, 512], bf16, tag="eT")
                    nc.scalar.activation(
                        out=eT[:, col0:], in_=s_ps[:, col0:], func=Exp, scale=scale
                    )
                    if p >= 0:
                        nc.vector.tensor_mul(
                            out=eT[:, col0:],
                            in0=eT[:, col0:],
                            in1=mk_sb[:, 0:ncol],
                        )
                    if j == 0:
                        nc.vector.tensor_copy(out=s["sacc"][:], in_=eT[:])
                    else:
                        nc.vector.tensor_add(
                            out=s["sacc"][:, col0:],
                            in0=s["sacc"][:, col0:],
                            in1=eT[:, col0:],
                        )
                    s["eTs"][j] = (eT, col0)

                def emit_attnv(s, j):
                    eT, col0 = s["eTs"].pop(j)
                    nc.tensor.matmul(
                        s["ao"][:, col0:],
                        lhsT=v_sb[:, b * KPB + j, ts(s["ht"], 128)],
                        rhs=eT[:, col0:],
                        start=(j == 0),
                        stop=(j == nk - 1),
                    )

                for j in range(nk):
                    for s in streams:
                        emit_scores(s, j)
                        if j >= LOOKAHEAD:
                            emit_attnv(s, j - LOOKAHEAD)
                        paced_pump()
                for j in range(max(0, nk - LOOKAHEAD), nk):
                    for s in streams:
                        emit_attnv(s, j)

                def make_finalize(s):
                    def finalize(b=b, m=m):
                        rps = psred.tile([128, 512], f32, tag="red")
                        nc.tensor.matmul(
                            rps[:],
                            lhsT=ones_sb[:],
                            rhs=s["sacc"][:],
                            start=True,
                            stop=True,
                        )
                        rsb = spool.tile([128, 512], f32, tag="rsb")
                        nc.vector.reciprocal_approx_fast(out=rsb[:], in_=rps[:])
                        # normalize in 128-column chunks: each downstream
                        # o_proj tile only reads its own 128 tokens, so it can
                        # start as soon as its chunk lands instead of waiting
                        # for the full 512-wide multiply
                        for c4 in range(4):
                            nc.vector.tensor_mul(
                                out=ao_sb[
                                    :, s["ht"], ds(b * S_ + m * 512 + c4 * 128, 128)
                                ],
                                in0=s["ao"][:, ts(c4, 128)],
                                in1=rsb[:, ts(c4, 128)],
                            )
                        if s["ht"] == HPC - 1 and post_group_hook is not None:
                            post_group_hook(m)

                    return finalize

                # first head's finalize lands here (the tail of the other
                # head's work hides its DVE burst); the last head's is
                # deferred into the next slot as before
                make_finalize(streams[0])()
                pending[0] = make_finalize(streams[1])

                # hard filler must be fully emitted before the next slot's
                # attention (which reads the q/k/v it produces)
                while hardq:
                    hardq.popleft()()

            # ---- program order ----
            emit_xg(0, interleave_wq=True)
            emit_late_consts()
            # dense first group: nothing can overlap it (everything depends
            # on x arriving)
            for job in qk_jobs:
                emit_qk_ftile(0, job)
            for tsub in range(TG // 128):
                emit_v_sub(0, tsub)
            emit_xg(1)
            if GPB > 1:
                # remaining rope-table blocks ride the gpsimd ring (idle
                # after the wq pieces) — on the sync ring they would delay
                # xg(2) behind them; needed by group-1 ftiles in slot 0
                nc.gpsimd.dma_start(out=cs2_sb[:, TG:], in_=cs2[:, TG:])
                nc.gpsimd.dma_start(out=sn2_sb[:, TG:], in_=sn2[:, TG:])
            # wo is first needed by o_proj of group 0 (~45us in): keep it out
            # of the congested early-DMA window
            nc.gpsimd.dma_start(
                out=wo_sb[:], in_=wo.rearrange("(o p) e -> p o e", p=128)
            )

            def queue_qkv_group(g):
                for job in qk_jobs:
                    hardq.append(lambda job=job: emit_qk_ftile(g, job))
                for tsub in range(TG // 128):
                    hardq.append(lambda tsub=tsub: emit_v_sub(g, tsub))

            def oproj_hook(b):
                def hook(m):
                    base = b * (T // 256)
                    for tt in range(base + 4 * m, base + 4 * m + 4):
                        softq.append(
                            lambda tt=tt, pool=None: emit_oproj_tile(tt, pool=pool)
                        )

                return hook

            NSLOT = B_ * NQG
            for si in range(NSLOT):
                b, m = divmod(si, NQG)
                # global qkv group consumed by slot si is g_next = si + 1
                g_next = si + 1
                if g_next < B_ * GPB:
                    queue_qkv_group(g_next)
                if g_next + 1 < B_ * GPB:
                    emit_xg(g_next + 1)
                emit_attention_slot(
                    b,
                    m,
                    post_group_hook=oproj_hook(b),
                    soft_reserve=2 if si == NSLOT - 1 else 0,
                    pump_first=2 if (si > 0 and m == 0) else 0,
                )

            do_pending()
            # tail drain: the last group's tiles (tail of softq) depend on the
            # just-emitted finalize; lead with the reserved independent tiles
            # and alternate PSUM pools (the scores pool is idle now)
            tiles = [softq.popleft() for _ in range(len(softq))]
            indep, dep = tiles[:-4], tiles[-4:]
            order = []
            while indep or dep:
                if indep:
                    order.append(indep.pop(0))
                if dep:
                    order.append(dep.pop(0))
            for ti, fn in enumerate(order):
                fn(pool=[None, psum, psacc][ti % 3])

    nc.compile()
    return nc


def _rope_tables(token_positions, S_):
    pos = np.asarray(token_positions).astype(np.float32)
    dim_id = np.arange(0, HEAD_DIM, 2, dtype=np.float32)
    inv_freq = np.power(np.float32(THETA), dim_id / np.float32(HEAD_DIM)).astype(
        np.float32
    )
    ang = (pos[None, :] / inv_freq[:, None]).astype(np.float32)  # [64, S]
    cos = np.cos(ang).astype(np.float32)
    sin = np.sin(ang).astype(np.float32)
    cs2 = np.concatenate([cos, cos], axis=0)  # [128, S]
    # pre-multiply sign layout: top half (x0 rows) gets +sin (feeds r1 after
    # the half-swap), bottom half (x1 rows) gets -sin (feeds r0)
    sn2 = np.concatenate([sin, -sin], axis=0)
    return np.ascontiguousarray(cs2), np.ascontiguousarray(sn2)


def _masks():
    # single lower-triangle table: for a diagonal tile at k-offset p*128 the
    # matmul already starts at column col0 = p*128, so the mask seen by the
    # surviving columns is always (k <= q - col0)
    kl = np.arange(128)[:, None]
    ql = np.arange(512)[None, :]
    return np.ascontiguousarray((kl <= ql).astype(np.float32).astype(BF16))


def _perm(n_heads):
    # within each 128-feature head block: evens then odds
    p = []
    for h in range(n_heads):
        base = h * HEAD_DIM
        p.extend(range(base, base + HEAD_DIM, 2))
        p.extend(range(base + 1, base + HEAD_DIM, 2))
    return np.array(p, dtype=np.int64)


def prepare_in_maps(x, token_positions, qkv_w, o_w, D=D_MODEL, S_=S, B_=B, ncores=NCORES):
    T = B_ * S_
    x = np.asarray(x, dtype=np.float32)
    qkv_w = np.asarray(qkv_w, dtype=np.float32)
    o_w = np.asarray(o_w, dtype=np.float32)

    xt = np.ascontiguousarray(x.reshape(T, D).T).astype(BF16)  # [D, T]
    cs2, sn2 = _rope_tables(token_positions, S_)
    cs2 = cs2.astype(BF16)
    sn2 = sn2.astype(BF16)
    mk = _masks()
    perm = _perm(HPC)

    in_maps = []
    for c in range(ncores):
        r0 = c * F
        qrows = qkv_w[r0 : r0 + F]
        krows = qkv_w[D + r0 : D + r0 + F]
        vrows = qkv_w[2 * D + r0 : 2 * D + r0 + F]
        wq_c = np.ascontiguousarray(qrows[perm].T).astype(BF16)  # [D, F]
        wk_c = np.ascontiguousarray(krows[perm].T).astype(BF16)
        wv_c = np.ascontiguousarray(vrows.T).astype(BF16)
        wo_c = np.ascontiguousarray(o_w[:, r0 : r0 + F].T).astype(BF16)  # [F, D]
        in_maps.append(
            {
                "xt": xt,
                "wq": wq_c,
                "wk": wk_c,
                "wv": wv_c,
                "wo": wo_c,
                "cs2": cs2,
                "sn2": sn2,
                "mk": mk,
            }
        )
    return in_maps


_CACHE = {}


def kernel_with_results(x, token_positions, qkv_w, o_w, trace=False, **kw):
    from concourse.bass_utils import run_bass_kernel_spmd

    if "nc" not in _CACHE:
        _CACHE["nc"] = build_module()
    nc = _CACHE["nc"]

    in_maps = prepare_in_maps(x, token_positions, qkv_w, o_w)
    res = run_bass_kernel_spmd(
        nc, in_maps, core_ids=list(range(NCORES)), trace=trace, **kw
    )
    acc = np.zeros((B * S, D_MODEL), dtype=np.float64)
    for r in res.results:
        acc += r["y"].astype(np.float64)
    return acc.astype(np.float32).reshape(B, S, D_MODEL), res


def kernel(x, token_positions, qkv_w, o_w):
    out, _ = kernel_with_results(x, token_positions, qkv_w, o_w)
    return out
